# revision 17
# baseline (speedup 1.0000x reference)
"""Trainium2 Bass kernel for nn_Program_4578435138231.

Sharding: pure data parallelism over the batch dim (262144 rows) across 8
NeuronCores; classifier weights replicated (pre-packed host-side).

Per core (R=32768 rows), rows are processed in chunks of 128*FC laid out
row-major in an SBUF out_tile (178 fp32 per row) so the final DMA is one
contiguous run per partition.

The 15-step scan runs with a hybrid layout:
  - batch-major [128, FC] column views for the argmax/select logic and
    output assembly (DVE + ACT),
  - an interleave-8 feature buffer that is PE-transposed each step into a
    16-group x 8-slot feature-major layout where the whole classifier
    (conv1 -> conv2 -> 4->32 dense -> 32->5 dense) runs as block-diagonal
    TensorEngine matmuls with biases folded into per-partition activation
    bias vectors; sigmoid output is PE-transposed back to batch-major.
"""

from contextlib import ExitStack

import numpy as np

import bass_rust
import concourse.bass as bass
import concourse.tile as tile
from concourse import mybir
from concourse.bass_utils import run_bass_kernel_spmd
from concourse.vector_clock import ScopedClock

# ---------------------------------------------------------------- constants
N_CORES = 8
B = 262144
R = B // N_CORES          # rows per core
FC = 128                  # rows per partition per chunk
CHUNK_ROWS = 128 * FC
N_CHUNKS = R // CHUNK_ROWS
NW8 = 8 * FC // 128       # fwd/back transpose windows per chunk (8)
NT = 8 * FC               # feature-major free size per chunk (1024)
STEPS = 15
SPEED = 5.0
NCOL = 178                # 18 state + 160 traj
F32 = mybir.dt.float32
OP = mybir.AluOpType
AF = mybir.ActivationFunctionType

# wconst columns (per-partition bias vectors / scalars)
C_C1B, C_C2B = 0, 1
C_B3 = 2                  # 2..5: relu3 bias for dense1 quarter q
C_B4 = 6                  # sigmoid bias (l2b at slot 8g+c)
NWC = 7

# lmats column offsets (packed lhsT matrices + identity)
L_ID = 0                  # identity [128,128]
L_C1 = 128                # conv1 lhsT [128 (8g+s), 80 (5g+o)]
L_C2 = L_C1 + 80          # conv2 lhsT [80 (5g+i), 64 (4g+o)]
L_D1 = L_C2 + 64          # dense1 lhsT x4 [64 (4g+k), 128 (8g+ol)]
L_D2 = L_D1 + 4 * 128     # dense2 lhsT x4 [128 (8g+ol), 128 (8g+c)]
NLM = L_D2 + 4 * 128


# --------------------------------------------------- tail-drain split patch
def _split_drain_and_barrier(self, tick_clock, wait_clock):
    nc = self.nc
    drain_inst = nc.sync.drain()
    wait_clock.add_sem_waits(
        drain_inst.ins, ScopedClock({None: tick_clock.global_clock})
    )
    si = drain_inst.ins.sync_info
    waits = list(si.on_wait) if si is not None else []
    if len(waits) > 1:
        updates = list(si.on_update) if si is not None else []
        drain_inst.ins.sync_info = bass_rust.SyncInfo(
            on_wait=[waits[0]], on_update=updates
        )
        for w in waits[1:]:
            extra = nc.sync.drain()
            extra.ins.sync_info = bass_rust.SyncInfo(on_wait=[w], on_update=[])
    nc.all_engine_barrier()
    assert self.sems is not None
    popped = nc._tile_sem_poison_stack.pop()
    assert popped is self._sem_poison
    nc.clear_and_free_semaphores(list(self.sems.allocated().values()))
    nc.all_engine_barrier()


tile.TileContext._drain_and_barrier = _split_drain_and_barrier


def _split_multiwait_instructions(nc):
    """This walrus build rejects >1 sync-wait per instruction; hoist extra
    waits onto same-engine NOPs inserted immediately before the owner."""
    for fn in nc.m.functions:
        for bb in fn.blocks:
            insts = bb.instructions
            i = 0
            while i < len(insts):
                inst = insts[i]
                si = getattr(inst, "sync_info", None)
                waits = list(si.on_wait) if si is not None else []
                if len(waits) > 1:
                    updates = list(si.on_update)
                    for w in waits[:-1]:
                        nop = mybir.InstNoOp(
                            name=nc.get_next_instruction_name(), ins=[], outs=[]
                        )
                        nop.engine = inst.engine
                        nop.sync_info = bass_rust.SyncInfo(
                            on_wait=[w], on_update=[]
                        )
                        nc.register_instruction(nop, overwrite=True)
                        insts.insert(i, nop)
                        i += 1
                    inst.sync_info = bass_rust.SyncInfo(
                        on_wait=[waits[-1]], on_update=updates
                    )
                i += 1


# ------------------------------------------------------------- bass program
def _emit_chunk(nc, tc, pools, xin, wc, lm, out_t):
    v = nc.vector
    s = nc.scalar
    pe = nc.tensor
    Fc = FC

    xin3 = xin.rearrange("p (j d) -> p j d", d=18)
    out3 = out_t.rearrange("p (j d) -> p j d", d=NCOL)

    def xcol(c):
        return xin3[:, :, c]

    def ocol(c):
        return out3[:, :, c]

    ident = lm[:, L_ID : L_ID + 128]

    work = pools["work"]
    fcur = work.tile([128, NT], F32, tag="fcur")
    fnxt = work.tile([128, NT], F32, tag="fnxt")
    tsb = work.tile([128, NT], F32, tag="tsb")
    h1T = work.tile([80, NT], F32, tag="h1T")
    h2T = work.tile([64, NT], F32, tag="h2T")
    h3T = [
        work.tile([128, NT], F32, tag=f"h3T{q}", name=f"h3T{q}")
        for q in range(4)
    ]
    pT = work.tile([128, NT], F32, tag="pT")
    pilv = work.tile([128, NT], F32, tag="pilv")
    mk = work.tile([128, 3 * Fc], F32, tag="mk")          # C, E, F (values)
    mku = work.tile([128, 3 * Fc], mybir.dt.uint8, tag="mku")  # A, B, D (preds)
    t0 = work.tile([128, Fc], F32, tag="t0")
    t1 = work.tile([128, Fc], F32, tag="t1")
    dxf = work.tile([128, Fc], F32, tag="dxf")
    dxe = work.tile([128, Fc], F32, tag="dxe")
    dxc = work.tile([128, Fc], F32, tag="dxc")
    stf = work.tile([128, Fc], F32, tag="stf")
    ste = work.tile([128, Fc], F32, tag="ste")

    psum = pools["psA"]

    fc3 = fcur.rearrange("p (j k) -> p j k", k=8)
    fn3 = fnxt.rearrange("p (j k) -> p j k", k=8)

    # ---- chunk init ----
    v.tensor_scalar_add(ocol(0), xcol(0), float(STEPS))       # s0 final
    s.copy(ocol(4), xcol(4))                                  # s4 final
    # dist0 -> traj0 col 0
    v.tensor_tensor(t0[:], xcol(1), xcol(3), OP.subtract)
    v.tensor_tensor(t0[:], t0[:], t0[:], OP.mult)
    v.tensor_tensor(t1[:], xcol(2), xcol(4), OP.subtract)
    v.tensor_tensor(t1[:], t1[:], t1[:], OP.mult)
    v.tensor_tensor(ocol(18 + 0), t0[:], t1[:], OP.add)
    # traj0 cols 1..9 = x[1..8,17]; cols 1..8 are contiguous in x
    v.tensor_copy(out3[:, :, 19:27], xin3[:, :, 1:9])
    s.copy(ocol(27), xcol(17))
    # feat ilv-8 buffer init: slots [s1,s2,s3,s4,s9,s17,pad,pad]
    v.tensor_copy(fc3[:, :, 0:4], xin3[:, :, 1:5])
    s.copy(fc3[:, :, 4], xcol(9))
    s.copy(fc3[:, :, 5], xcol(17))
    nc.gpsimd.memset(fc3[:, :, 6:8], 0.0)
    nc.gpsimd.memset(fn3[:, :, 6:8], 0.0)
    s.copy(fn3[:, :, 3], xcol(4))                             # s4 static

    cur3, nxt3 = fc3, fn3
    cur, nxt = fcur, fnxt
    for t in range(1, STEPS + 1):
        last = t == STEPS
        tb = 18 + 10 * t

        # ---- fwd transpose: cur ilv-8 -> feature-major [16g x 8slot, NT]
        tps = psum.tile([128, 512], F32, tag="ps_t")
        tps2 = psum.tile([128, 512], F32, tag="ps_t")
        for w in range(NW8):
            dst = (tps if w < 4 else tps2)[:, (w % 4) * 128 : (w % 4 + 1) * 128]
            pe.transpose(dst, cur[:, w * 128 : (w + 1) * 128], ident)
        v.tensor_copy(tsb[:, 0:512], tps[:])
        v.tensor_copy(tsb[:, 512:1024], tps2[:])

        # ---- conv1 ----
        c1ps = pools["psB"].tile([80, NT], F32, tag="ps_m")
        lhs = lm[:, L_C1 : L_C1 + 80]
        pe.matmul(c1ps[:, 0:512], lhs, tsb[:, 0:512])
        pe.matmul(c1ps[:, 512:1024], lhs, tsb[:, 512:1024])
        s.activation(h1T[:], c1ps[:], AF.Relu, bias=wc[:80, C_C1B : C_C1B + 1])
        # ---- conv2 ----
        c2ps = pools["psB"].tile([64, NT], F32, tag="ps_m")
        lhs = lm[0:80, L_C2 : L_C2 + 64]
        pe.matmul(c2ps[:, 0:512], lhs, h1T[:, 0:512])
        pe.matmul(c2ps[:, 512:1024], lhs, h1T[:, 512:1024])
        s.activation(h2T[:], c2ps[:], AF.Relu, bias=wc[:64, C_C2B : C_C2B + 1])
        # ---- dense1 (4 quarters of 8 outs x 16 groups) + relu3 ----
        for q in range(4):
            dps = psum.tile([128, NT], F32, tag="d1ps")
            lhs = lm[0:64, L_D1 + q * 128 : L_D1 + (q + 1) * 128]
            pe.matmul(dps[:, 0:512], lhs, h2T[:, 0:512])
            pe.matmul(dps[:, 512:1024], lhs, h2T[:, 512:1024])
            bias = wc[:, C_B3 + q : C_B3 + q + 1]
            if q % 2 == 0:
                s.activation(h3T[q][:], dps[:], AF.Relu, bias=bias)
            else:
                # relu3 on DVE: (psum + bias) max 0 in one fused op
                v.tensor_scalar(
                    h3T[q][:], dps[:], bias, 0.0, OP.add, OP.max
                )
        # ---- dense2 (accumulate 4 K-chunks) + sigmoid ----
        d2ps = pools["psB"].tile([128, NT], F32, tag="ps_m")
        for q in range(4):
            lhs = lm[:, L_D2 + q * 128 : L_D2 + (q + 1) * 128]
            pe.matmul(
                d2ps[:, 0:512], lhs, h3T[q][:, 0:512],
                start=(q == 0), stop=(q == 3),
            )
            pe.matmul(
                d2ps[:, 512:1024], lhs, h3T[q][:, 512:1024],
                start=(q == 0), stop=(q == 3),
            )
        s.activation(pT[:], d2ps[:], AF.Sigmoid, bias=wc[:, C_B4 : C_B4 + 1])

        # ---- back transpose: pT -> batch-major p ilv-8, staged to SBUF ----
        pilv_a = psum.tile([128, 512], F32, tag="ps_t")
        pilv_b = psum.tile([128, 512], F32, tag="ps_t")
        for w in range(NW8):
            dst = (pilv_a if w < 4 else pilv_b)[
                :, (w % 4) * 128 : (w % 4 + 1) * 128
            ]
            pe.transpose(dst, pT[:, w * 128 : (w + 1) * 128], ident)
        v.tensor_copy(pilv[:, 0:512], pilv_a[:])
        v.tensor_copy(pilv[:, 512:1024], pilv_b[:])
        pv3 = pilv.rearrange("p (j k) -> p j k", k=8)

        # traj cols 5..9 = p0..p4 (one strided copy)
        s.copy(out3[:, :, tb + 5 : tb + 10], pv3[:, :, 0:5])
        if last:
            v.tensor_copy(out3[:, :, 5:9], pv3[:, :, 0:4])
            s.copy(out3[:, :, 17], pv3[:, :, 4])
        # p4 -> next feat slot 5 (s17)
        s.copy(nxt3[:, :, 5], pv3[:, :, 4])

        # ---- masks from p differences: M = [p_hi > p_lo] ----
        # a,b,c,d,e,f pair (hi,lo) indices; A,B,D as uint8 predicate masks,
        # C,E,F as fp32 value masks.
        def mkf(i):
            return mk[:, i * Fc : (i + 1) * Fc]

        def mki(i):
            return mku[:, i * Fc : (i + 1) * Fc]

        for dst, (hi, lo) in [
            (mki(0), (1, 0)),   # A
            (mki(1), (2, 0)),   # B
            (mkf(0), (3, 0)),   # C
            (mki(2), (2, 1)),   # D
            (mkf(1), (3, 1)),   # E
            (mkf(2), (3, 2)),   # F
        ]:
            v.tensor_tensor(dst, pv3[:, :, hi], pv3[:, :, lo], OP.is_gt)
        if last:
            for i, (hi, lo) in enumerate(
                [(1, 0), (2, 0), (3, 0), (2, 1), (3, 1), (3, 2)]
            ):
                v.tensor_tensor(
                    out3[:, :, 11 + i], pv3[:, :, hi], pv3[:, :, lo],
                    OP.subtract,
                )
        MA, MB, MD = mki(0), mki(1), mki(2)
        MC, ME, MF = mkf(0), mkf(1), mkf(2)

        # ---- dx / st via predicated overwrites ----
        v.tensor_scalar_mul(dxc[:], MC, SPEED)                    # dx_c
        v.tensor_scalar_mul(dxf[:], MF, SPEED)                    # dx_f
        v.tensor_scalar(dxe[:], ME, 2.0 * SPEED, -SPEED, OP.mult, OP.add)
        v.copy_predicated(dxc[:], MB, dxf[:])                     # dx_b
        v.copy_predicated(dxe[:], MD, dxf[:])                     # dx_d
        v.copy_predicated(dxc[:], MA, dxe[:])                     # dx
        v.tensor_tensor(nxt3[:, :, 0], cur3[:, :, 0], dxc[:], OP.add)  # s1'
        v.tensor_scalar_mul(t0[:], MC, 3.0)                       # st_c
        v.tensor_scalar_add(stf[:], MF, 2.0)                      # st_f
        v.tensor_scalar(ste[:], ME, 2.0, 1.0, OP.mult, OP.add)    # st_e
        v.copy_predicated(t0[:], MB, stf[:])                      # st_b
        v.copy_predicated(ste[:], MD, stf[:])                     # st_d
        v.copy_predicated(t0[:], MA, ste[:])                      # st
        v.tensor_copy(nxt3[:, :, 4], t0[:])
        if last:
            s.copy(ocol(9), t0[:])

        # ---- deterministic updates ----
        v.tensor_scalar_add(nxt3[:, :, 1], cur3[:, :, 1], SPEED)  # s2'
        v.tensor_scalar_add(nxt3[:, :, 2], cur3[:, :, 2], SPEED)  # s3'
        # traj cols 1..4 = [s1',s2',s3',s4] (slots 0..3 of nxt)
        v.tensor_copy(out3[:, :, tb + 1 : tb + 5], nxt3[:, :, 0:4])
        if last:
            v.tensor_copy(out3[:, :, 1:5], nxt3[:, :, 0:4])
        # dist -> traj col 0
        v.tensor_tensor(t0[:], nxt3[:, :, 0], nxt3[:, :, 2], OP.subtract)
        v.tensor_tensor(t0[:], t0[:], t0[:], OP.mult)
        v.tensor_tensor(t1[:], nxt3[:, :, 1], nxt3[:, :, 3], OP.subtract)
        v.tensor_tensor(t1[:], t1[:], t1[:], OP.mult)
        v.tensor_tensor(ocol(tb + 0), t0[:], t1[:], OP.add)
        if last:
            v.tensor_tensor(ocol(10), t0[:], t1[:], OP.add)

        cur3, nxt3 = nxt3, cur3
        cur, nxt = nxt, cur


def build_nc():
    nc = bass.Bass()
    x = nc.declare_dram_parameter("x", [R, 18], F32, isOutput=False)
    wc_d = nc.declare_dram_parameter("wconst", [128, NWC], F32, isOutput=False)
    lm_d = nc.declare_dram_parameter("lmats", [128, NLM], F32, isOutput=False)
    out = nc.declare_dram_parameter("out", [R, NCOL], F32, isOutput=True)

    xr = x[:].rearrange("(c p j) d -> c p (j d)", c=N_CHUNKS, p=128)
    outr = out[:].rearrange("(c p j) d -> c p (j d)", c=N_CHUNKS, p=128)

    with tile.TileContext(nc) as tc:
        with ExitStack() as ctx:
            pools = {
                "io": ctx.enter_context(tc.tile_pool(name="io", bufs=2)),
                "w": ctx.enter_context(tc.tile_pool(name="w", bufs=1)),
                "work": ctx.enter_context(tc.tile_pool(name="work", bufs=1)),
                "out": ctx.enter_context(tc.tile_pool(name="out", bufs=1)),
                "psA": ctx.enter_context(
                    tc.tile_pool(name="psA", bufs=2, space="PSUM")
                ),
                "psB": ctx.enter_context(
                    tc.tile_pool(name="psB", bufs=1, space="PSUM")
                ),
            }
            wc = pools["w"].tile([128, NWC], F32)
            nc.sync.dma_start(wc[:], wc_d[:])
            lmt = pools["w"].tile([128, NLM], F32)
            nc.sync.dma_start(lmt[:], lm_d[:])
            for c in range(N_CHUNKS):
                xin = pools["io"].tile([128, FC * 18], F32, tag="xin")
                nc.sync.dma_start(xin[:], xr[c])
                out_t = pools["out"].tile([128, FC * NCOL], F32, tag="out_t")
                _emit_chunk(nc, tc, pools, xin[:], wc[:], lmt[:], out_t[:])
                nc.sync.dma_start(outr[c], out_t[:])
    _split_multiwait_instructions(nc)
    return nc


_NC_CACHE = None


def _get_nc():
    global _NC_CACHE
    if _NC_CACHE is None:
        _NC_CACHE = build_nc()
    return _NC_CACHE


def _make_consts(c1w, c1b, c2w, c2b, l1w, l1b, l2w, l2b):
    c1w = np.asarray(c1w, np.float32)
    c2w = np.asarray(c2w, np.float32)
    l1w = np.asarray(l1w, np.float32)
    l2w = np.asarray(l2w, np.float32)

    wc = np.zeros((128, NWC), np.float32)
    wc[:, C_C1B] = np.float32(c1b[0])
    wc[:, C_C2B] = np.float32(c2b[0])
    for p in range(128):
        g, sl = divmod(p, 8)
        for q in range(4):
            wc[p, C_B3 + q] = l1b[8 * q + sl]
        wc[p, C_B4] = l2b[sl] if sl < 5 else 0.0

    lm = np.zeros((128, NLM), np.float32)
    lm[:, L_ID : L_ID + 128] = np.eye(128, dtype=np.float32)
    # conv1: [8g+s, 5g+o]: h1_o = c1w0*f_o + c1w1*f_{o+1} (+c1b via bias)
    for g in range(16):
        for o in range(5):
            lm[8 * g + o, L_C1 + 5 * g + o] = c1w[0]
            lm[8 * g + o + 1, L_C1 + 5 * g + o] = c1w[1]
    # conv2: [5g+i, 4g+o]: h2_o = c2w0*h1_o + c2w1*h1_{o+1}
    for g in range(16):
        for o in range(4):
            lm[5 * g + o, L_C2 + 4 * g + o] = c2w[0]
            lm[5 * g + o + 1, L_C2 + 4 * g + o] = c2w[1]
    # dense1 quarter q: [4g+k, 8g+ol] = l1w[k, 8q+ol]
    for q in range(4):
        for g in range(16):
            for k in range(4):
                for ol in range(8):
                    lm[4 * g + k, L_D1 + q * 128 + 8 * g + ol] = l1w[k, 8 * q + ol]
    # dense2 quarter q: [8g+ol, 8g+c] = l2w[8q+ol, c] (c<5)
    for q in range(4):
        for g in range(16):
            for ol in range(8):
                for c in range(5):
                    lm[8 * g + ol, L_D2 + q * 128 + 8 * g + c] = l2w[8 * q + ol, c]
    return np.ascontiguousarray(wc), np.ascontiguousarray(lm)


def kernel(x, c1w, c1b, c2w, c2b, l1w, l1b, l2w, l2b):
    x = np.asarray(x, np.float32)
    wc, lm = _make_consts(c1w, c1b, c2w, c2b, l1w, l1b, l2w, l2b)
    nc = _get_nc()
    in_maps = [
        {
            "x": np.ascontiguousarray(x[i * R : (i + 1) * R]),
            "wconst": wc,
            "lmats": lm,
        }
        for i in range(N_CORES)
    ]
    res = run_bass_kernel_spmd(nc, in_maps, list(range(N_CORES)))
    return np.concatenate([res.results[i]["out"] for i in range(N_CORES)], axis=0)
